# revision 37
# baseline (speedup 1.0000x reference)
"""Trainium2 Bass kernel for the GCM (global context module) problem.

Computation per batch sample b (x_b = x[b] viewed as [C=512, HW=9216]):
    x1 = w1 @ x_b                      [128, HW]
    x2 = w2 @ x_b                      [256, HW]
    v  = softmax_all(x1 @ x2^T)        [128, 256]  (softmax over all 32768)
    n  = relu(v + w3 @ v)              [128, 256]
    z  = w4 @ n^T                      [256, 128]
    W  = w5 @ z                        [512, 128]  (collapses y/conv5: w5@(z@x1) == (w5@z)@x1)
    out = x_b + W @ x1                 [512, HW]

Sharding: data-parallel over batch, one sample per NeuronCore (8 cores).

v3 strategy: fp8 (e4m3) DoubleRow phase-1, bf16 residual/output.
  - x uploaded twice: fp8 plane (4.7 MB, feeds the phase-1 GEMMs) and
    bf16 plane (9.4 MB, feeds the residual add). Output bf16 (9.4 MB).
  - Phase 1: [x1T|x2T] = xT @ [w1T|w2T] as fp8 DoubleRow matmuls
    (K=256 per pass -> 2 passes instead of 4; 2 elem/cycle). Weights
    scaled by 64 so small weights stay in e4m3 normal range; the
    stored xts tiles are 64*[x1T|x2T] in fp8, v_psum = 4096*v.
    v accumulates via DoubleRow on subtile pairs (K=256 of hw).
  - Softmax: exp(v'/4096 - max'/4096) on ACT with fused row sums
    (accum_out); 1/S folded into the z copy, 1/64 into the W copy.
  - Phase 2: x1 reused via PE transposes of stored fp8 x1T tiles.
    Per 512-tile: oc0/1 through a 2-bank PSUM tile + one fused DVE
    add (residual from contiguous bf16 x block tile), oc2/3 add the
    residual on the PE (identity matmul) + one fused ACT copy.
    Output staged [128, 4, 1536] bf16 per block, last block DMA'd
    per-tile to shorten the tail.

Numerics: the softmax is a hard argmax (top-2 gap ~90 vs fp8-induced
v noise ~1.4 std) so fp8 cannot flip it; x_res carries ~5% fp8 error
on an absmax-0.047 branch (tolerance 0.108); bf16 passthrough of x
costs <= 0.016. Measured end-to-end ~3e-2 max abs diff vs the
0.108 gate.
"""

import numpy as np
import ml_dtypes

import concourse.bass as bass
import concourse.tile as tile
from concourse import bacc, mybir, bass_isa
from concourse.bass_utils import run_bass_kernel_spmd

F32 = mybir.dt.float32
BF16 = mybir.dt.bfloat16
FP8 = mybir.dt.float8e4
DR = mybir.MatmulPerfMode.DoubleRow
AX = mybir.AxisListType
AL = mybir.AluOpType
AF = mybir.ActivationFunctionType

N_CORES = 8
C = 512
H = W_IMG = 96
HW = H * W_IMG          # 9216
CK = C // 128           # 4 chunks of channels
NBLK = 6                # x blocks along hw
BLK = HW // NBLK        # 1536
NSUB = HW // 128        # 72 phase-1 subtiles
NPAIR = NSUB // 2       # 36 subtile pairs (DoubleRow v)
SUB_PER_BLK = BLK // 128
NT = HW // 512          # 18 phase-2 tiles
C4 = C // 4             # 128
C2 = C // 2             # 256
KM = C4 + C2            # 384 = concat(x1T, x2T) free size

WSCALE = 64.0           # fp8 weight scale (w12 * 64)
VSCALE = WSCALE * WSCALE  # v_psum = VSCALE * v


def _emit(ctx, tc, aps, use_bias):
    nc = tc.nc
    xq_d = aps["xq"]
    xb_d = aps["xb"]
    w12q_d = aps["w12q"]
    w3t_d = aps["w3t"]
    w4t_d = aps["w4t"]
    w5t_d = aps["w5t"]
    out_d = aps["out"]

    consts = ctx.enter_context(tc.tile_pool(name="consts", bufs=1))

    # Phase-1-critical DMAs go first on the sync queue (identb for PE
    # warmup, w12q + x block 0 for the first matmuls); the other consts
    # issue from the scalar queue so they don't delay them.
    identb = consts.tile([128, 128], BF16, tag="identb")
    nc.sync.dma_start(out=identb[:], in_=aps["identb"][:, :])

    # w12q: fp8, 64x, paired for DoubleRow: [128, 2, 384] per c-chunk pair
    w12 = []
    for q in range(2):
        t = consts.tile([128, 2, KM], FP8, tag=f"w12_{q}")
        for i in range(2):
            r0 = (2 * q + i) * 128
            nc.sync.dma_start(out=t[:, i, :], in_=w12q_d[r0 : r0 + 128, :])
        w12.append(t)
    ident8 = consts.tile([128, 128], FP8, tag="ident8")
    nc.scalar.dma_start(out=ident8[:], in_=aps["ident8"][:, :])
    w3t = consts.tile([128, 128], BF16, tag="w3t")
    nc.scalar.dma_start(out=w3t[:], in_=w3t_d[:, :])
    w4t = []
    for q in range(2):
        t = consts.tile([128, C2], BF16, tag=f"w4t_{q}")
        nc.scalar.dma_start(out=t[:], in_=w4t_d[q * 128 : (q + 1) * 128, :])
        w4t.append(t)
    w5t = []
    for q in range(2):
        t = consts.tile([128, C], BF16, tag=f"w5t_{q}")
        nc.scalar.dma_start(out=t[:], in_=w5t_d[q * 128 : (q + 1) * 128, :])
        w5t.append(t)

    bias_t = {}
    if use_bias:
        b12row_d = aps["b12row"]  # already scaled by WSCALE on host
        b3_d, b4_d, b5_d = aps["b3c"], aps["b4c"], aps["b5c"]
        brow1 = consts.tile([1, KM], F32, tag="brow1")
        nc.sync.dma_start(out=brow1[:], in_=b12row_d[:, :])
        brow = consts.tile([128, KM], F32, tag="brow")
        nc.gpsimd.partition_broadcast(brow[:], brow1[:])
        bias_t["brow"] = brow
        b3 = consts.tile([128, 1], F32, tag="b3")
        nc.sync.dma_start(out=b3[:], in_=b3_d[:, :])
        bias_t["b3"] = b3
        b4 = []
        for q in range(2):
            t = consts.tile([128, 1], F32, tag=f"b4_{q}")
            nc.sync.dma_start(out=t[:], in_=b4_d[q * 128 : (q + 1) * 128, :])
            b4.append(t)
        bias_t["b4"] = b4
        b5 = []
        for oc in range(CK):
            t = consts.tile([128, 1], F32, tag=f"b5_{oc}")
            nc.sync.dma_start(out=t[:], in_=b5_d[oc * 128 : (oc + 1) * 128, :])
            b5.append(t)
        bias_t["b5"] = b5

    # ---- x resident in SBUF ----
    # fp8 plane (phase-1 stationary operands) as one [128, 4, HW] tile,
    # the first 1536 hw cols split into [128,512] pieces for early
    # compute start. bf16 plane (residual) as one tile, 4 descriptors on
    # the otherwise-idle gpsimd DGE queue (it's only needed in phase 2).
    # Per-block tiles: dep tracking is per-tile for DMA writers, so one
    # big tile would stall the first matmul on the whole upload.
    xqpool = ctx.enter_context(tc.tile_pool(name="xq", bufs=1))
    xq = {}
    for b in range(NBLK):
        xq[b] = xqpool.tile([128, CK, BLK], FP8, tag=f"xq_{b}", name=f"xq_{b}")
    for p in range(BLK // 512):
        for c in range(CK):
            nc.sync.dma_start(
                out=xq[0][:, c, p * 512 : (p + 1) * 512],
                in_=xq_d[c * 128 : (c + 1) * 128, p * 512 : (p + 1) * 512],
            )
    for b in range(1, NBLK):
        for c in range(CK):
            nc.sync.dma_start(
                out=xq[b][:, c, :],
                in_=xq_d[c * 128 : (c + 1) * 128, b * BLK : (b + 1) * BLK],
            )
    # bf16 plane issued on the SAME sync queue AFTER all fp8 descriptors:
    # the DMA rings serve in issue order, so the phase-1-critical fp8
    # data is never starved by this 9.4 MB stream (it's only needed in
    # phase 2, ~40us later).
    xbpool = ctx.enter_context(tc.tile_pool(name="xb", bufs=1))
    xbt = xbpool.tile([128, CK, HW], BF16, tag="xb", name="xb")
    for b in range(NBLK):
        for c in range(CK):
            nc.sync.dma_start(
                out=xbt[:, c, b * BLK : (b + 1) * BLK],
                in_=xb_d[c * 128 : (c + 1) * 128, b * BLK : (b + 1) * BLK],
            )

    sm = ctx.enter_context(tc.tile_pool(name="sm", bufs=1))
    # persistent hw-major 64*[x1T|x2T] fp8 tiles, two subtiles per tile
    # (DoubleRow v contracts over 256 hw at a time)
    xtsp = ctx.enter_context(tc.tile_pool(name="xts", bufs=1))
    xts2 = [
        xtsp.tile([128, 2, KM], FP8, tag=f"xts_{j}", name=f"xts_{j}")
        for j in range(NPAIR)
    ]
    # k-major 64*x1 fp8 tiles, two phase-2 tiles per SBUF tile
    x1pool = ctx.enter_context(tc.tile_pool(name="x1", bufs=1))
    x1sb = [
        x1pool.tile([128, 1024], FP8, tag=f"x1_{k}", name=f"x1_{k}")
        for k in range(NT // 2)
    ]

    e = sm.tile([128, C2], BF16, tag="e")
    s1 = sm.tile([128, 1], F32, tag="s1")

    # ---- phase 1: x12T tiles + v accumulation; x1 transposes; softmax ----
    with (
        tc.tile_pool(name="psC", bufs=1, space="PSUM") as psC,
        tc.tile_pool(name="psA", bufs=3, space="PSUM") as psA,
        tc.tile_pool(name="vps", bufs=1, space="PSUM") as vps,
    ):
        v_ps = vps.tile([128, C2], F32, tag="v")

        # Warm the PE HAM clock-gate during the initial x-DMA window so
        # real phase-1 matmuls start at full clock. Targets the v bank
        # (v's first real matmul has start=True, which resets it).
        for _ in range(30):
            nc.tensor.matmul(
                v_ps[:, 0:128], identb[:], identb[:], start=True, stop=True
            )

        def emit_v(j):
            nc.tensor.matmul(
                v_ps[:],
                xts2[j][:, :, 0:C4],
                xts2[j][:, :, C4:KM],
                start=(j == 0),
                stop=(j == NPAIR - 1),
                perf_mode=DR,
            )

        def emit_tr_group(kk):
            # 8 transposes (4 xts2 pairs -> one k-major [128,1024] x1
            # tile) into one PSUM bank, then one fused copy to SBUF.
            # fp8 PE transposes must write with element step 2 (HW
            # quirk); the copy reads the same strided view.
            pc = psC.tile([128, 2048], FP8, tag="x1T", name=f"x1T_{kk}")
            for j in range(8):
                s = 8 * kk + j
                nc.tensor.transpose(
                    pc[:, j * 256 : (j + 1) * 256 : 2],
                    xts2[s // 2][:, s % 2, 0:C4],
                    ident8[:],
                )
            if kk % 2 == 0:
                nc.scalar.copy(x1sb[kk][:], pc[:, 0:2048:2])
            else:
                nc.vector.tensor_copy(x1sb[kk][:], pc[:, 0:2048:2])

        # psA tiles hold a PAIR of subtiles ([128,2,512] = 2 banks, 384
        # used per bank) so each pair drains with ONE fused copy,
        # alternating ACT/DVE (per-op fixed cost dominates these engines).
        # Every 4th pair, the finished x1T tiles are transposed back to
        # k-major in the PE stream (phase-1 PE has slack vs the copies).
        SKEW = 2  # pairs
        pend = []
        for j in range(NPAIR):
            b, poff = divmod(j, SUB_PER_BLK // 2)
            ps = psA.tile([128, 2, 512], F32, tag="xts")
            for i in range(2):
                off = (2 * poff + i) * 128
                for q in range(2):
                    nc.tensor.matmul(
                        ps[:, i, 0:KM],
                        xq[b][:, 2 * q : 2 * q + 2, off : off + 128],
                        w12[q][:],
                        start=(q == 0),
                        stop=(q == 1),
                        perf_mode=DR,
                    )
            if use_bias:
                for i in range(2):
                    nc.vector.tensor_tensor(
                        xts2[j][:, i, :], ps[:, i, 0:KM],
                        bias_t["brow"][:], op=AL.add,
                    )
            elif j % 2 == 0:
                nc.scalar.copy(xts2[j][:], ps[:, :, 0:KM])
            else:
                nc.vector.tensor_copy(xts2[j][:], ps[:, :, 0:KM])
            pend.append(j)
            if len(pend) > SKEW:
                emit_v(pend.pop(0))
            if j % 4 == 3:
                emit_tr_group(j // 4)
        while pend:
            emit_v(pend.pop(0))

        # ---- softmax over all 32768 entries of v (unnormalized exp) ----
        # v_ps holds VSCALE*v; fold 1/VSCALE into the exp scale/bias.
        m1 = sm.tile([128, 1], F32, tag="m1")
        nc.vector.tensor_reduce(m1[:], v_ps[:], axis=AX.X, op=AL.max)
        mall = sm.tile([128, 1], F32, tag="mall")
        nc.gpsimd.partition_all_reduce(mall[:], m1[:], 128, bass_isa.ReduceOp.max)
        negm = sm.tile([128, 1], F32, tag="negm")
        nc.vector.tensor_scalar_mul(negm[:], mall[:], -1.0 / VSCALE)
        nc.scalar.activation(
            e[:], v_ps[:], AF.Exp, bias=negm[:], scale=1.0 / VSCALE,
            accum_out=s1[:],
        )

    sall = sm.tile([128, 1], F32, tag="sall")
    nc.gpsimd.partition_all_reduce(sall[:], s1[:], 128, bass_isa.ReduceOp.add)
    sinv = sm.tile([128, 1], F32, tag="sinv")
    nc.vector.reciprocal(sinv[:], sall[:])

    wt = sm.tile([128, C], BF16, tag="wt")
    if True:
        with tc.tile_pool(name="psB", bufs=1, space="PSUM") as psB:
            # ---- small chain: conv3+relu, n^T, z (1/S folded), W^T ----
            if use_bias:
                en = sm.tile([128, C2], BF16, tag="en")
                nc.vector.tensor_scalar_mul(en[:], e[:], sinv[:])
                esrc = en
            else:
                esrc = e
            # n = relu(e + w3@e): the +e rides on the PE as an identity
            # matmul into the same PSUM bank, relu is a single ACT op.
            ps3 = psB.tile([128, C2], F32, tag="ps3")
            nc.tensor.matmul(ps3[:], w3t[:], esrc[:], start=True, stop=False)
            nc.tensor.matmul(ps3[:], identb[:], esrc[:], start=False, stop=True)
            nsb = sm.tile([128, C2], BF16, tag="nsb")
            if use_bias:
                nc.scalar.activation(
                    nsb[:], ps3[:], AF.Relu, bias=bias_t["b3"][:]
                )
            else:
                nc.scalar.activation(nsb[:], ps3[:], AF.Relu)

            pT = psB.tile([128, C2], BF16, tag="pT")
            for q in range(2):
                nc.tensor.transpose(
                    pT[:, q * 128 : (q + 1) * 128],
                    nsb[:, q * 128 : (q + 1) * 128],
                    identb[:],
                )
            ntt = sm.tile([128, C2], BF16, tag="ntt")
            nc.scalar.copy(ntt[:], pT[:])
            nts = [ntt[:, 0:128], ntt[:, 128:256]]

            pz = psB.tile([128, C2], F32, tag="pz")
            for mc in range(2):
                pzs = pz[:, mc * 128 : (mc + 1) * 128]
                for q in range(2):
                    nc.tensor.matmul(
                        pzs,
                        w4t[q][:, mc * 128 : (mc + 1) * 128],
                        nts[q],
                        start=(q == 0),
                        stop=(q == 1),
                    )
            zt = sm.tile([128, C2], BF16, tag="zt")
            zs = [zt[:, 0:128], zt[:, 128:256]]
            if use_bias:
                for mc in range(2):
                    nc.scalar.add(
                        zs[mc], pz[:, mc * 128 : (mc + 1) * 128],
                        bias_t["b4"][mc][:],
                    )
            else:
                # fold softmax 1/S here: W = w5 @ (z/S), one fused op
                nc.vector.tensor_scalar_mul(zt[:], pz[:], sinv[:])

            pW = psB.tile([128, C], F32, tag="pW")
            for mc in range(2):
                nc.tensor.matmul(
                    pW[:], zs[mc], w5t[mc][:], start=(mc == 0), stop=(mc == 1)
                )
            # 1/WSCALE cancels the 64x in the fp8 x1 tiles
            nc.scalar.activation(wt[:], pW[:], AF.Copy, scale=1.0 / WSCALE)

        # ---- phase 2: x_res = W @ x1, residual, staged DMA out ----
        # oc0/1: 2-bank PSUM tile + one fused DVE add (bf16 x residual).
        # oc2/3: residual via PE identity matmul + one fused ACT copy.
        with (
            tc.tile_pool(name="psD", bufs=2, space="PSUM") as psD,
            tc.tile_pool(name="psE", bufs=2, space="PSUM") as psE,
            tc.tile_pool(name="outp", bufs=4) as outp,
        ):
            stage = None
            for t in range(NT):
                g, ti = divmod(t, 3)
                off = ti * 512  # offset within block g (BLK==1536==3*512)
                hw0 = t * 512
                k, half = divmod(t, 2)
                x1v = x1sb[k][:, half * 512 : (half + 1) * 512]
                if ti == 0:
                    stage = outp.tile(
                        [128, CK, BLK], BF16, tag="st", name=f"st_{g}"
                    )
                pr = psD.tile([128, 2, 512], F32, tag="pr")
                for oc in range(2):
                    nc.tensor.matmul(
                        pr[:, oc, :],
                        wt[:, oc * 128 : (oc + 1) * 128],
                        x1v,
                        start=True,
                        stop=True,
                    )
                pe = psE.tile([128, 2, 512], F32, tag="pe")
                for oc in range(2, 4):
                    pes = pe[:, oc - 2, :]
                    nc.tensor.matmul(
                        pes, wt[:, oc * 128 : (oc + 1) * 128], x1v,
                        start=True, stop=False,
                    )
                    nc.tensor.matmul(
                        pes, identb[:], xbt[:, oc, hw0 : hw0 + 512],
                        start=False, stop=True,
                    )
                if use_bias:
                    for oc in range(2):
                        nc.vector.scalar_tensor_tensor(
                            stage[:, oc, off : off + 512],
                            pr[:, oc, :],
                            bias_t["b5"][oc][:],
                            xbt[:, oc, hw0 : hw0 + 512],
                            op0=AL.add,
                            op1=AL.add,
                        )
                    for oc in range(2, 4):
                        nc.scalar.add(
                            stage[:, oc, off : off + 512],
                            pe[:, oc - 2, :],
                            bias_t["b5"][oc][:],
                        )
                else:
                    nc.vector.tensor_tensor(
                        stage[:, 0:2, off : off + 512],
                        pr[:],
                        xbt[:, 0:2, hw0 : hw0 + 512],
                        op=AL.add,
                    )
                    nc.scalar.copy(stage[:, 2:4, off : off + 512], pe[:])
                # one descriptor per block via a [128, 4, win] dram view
                # (dim0 = partition, dim1 = oc chunk); last block DMA'd
                # per tile to shorten the pipeline-drain tail.
                outv = out_d.rearrange("(o p) w -> p o w", o=CK)
                if g == NBLK - 1:
                    hw1 = g * BLK + off
                    nc.gpsimd.dma_start(
                        out=outv[:, :, hw1 : hw1 + 512],
                        in_=stage[:, :, off : off + 512],
                    )
                elif ti == 2:
                    nc.gpsimd.dma_start(
                        out=outv[:, :, g * BLK : (g + 1) * BLK],
                        in_=stage[:],
                    )


def _build(use_bias):
    nc = bacc.Bacc("TRN2", target_bir_lowering=False, debug=False, num_devices=N_CORES)
    aps = {
        "xq": nc.dram_tensor("xq", [C, HW], FP8, kind="ExternalInput").ap(),
        "xb": nc.dram_tensor("xb", [C, HW], BF16, kind="ExternalInput").ap(),
        "w12q": nc.dram_tensor("w12q", [C, KM], FP8, kind="ExternalInput").ap(),
        "w3t": nc.dram_tensor("w3t", [C4, C4], BF16, kind="ExternalInput").ap(),
        "w4t": nc.dram_tensor("w4t", [C2, C2], BF16, kind="ExternalInput").ap(),
        "w5t": nc.dram_tensor("w5t", [C2, C], BF16, kind="ExternalInput").ap(),
        "identb": nc.dram_tensor(
            "identb", [128, 128], BF16, kind="ExternalInput"
        ).ap(),
        "ident8": nc.dram_tensor(
            "ident8", [128, 128], FP8, kind="ExternalInput"
        ).ap(),
        "out": nc.dram_tensor("out", [C, HW], BF16, kind="ExternalOutput").ap(),
    }
    if use_bias:
        aps["b12row"] = nc.dram_tensor(
            "b12row", [1, KM], F32, kind="ExternalInput"
        ).ap()
        aps["b3c"] = nc.dram_tensor("b3c", [C4, 1], F32, kind="ExternalInput").ap()
        aps["b4c"] = nc.dram_tensor("b4c", [C2, 1], F32, kind="ExternalInput").ap()
        aps["b5c"] = nc.dram_tensor("b5c", [C, 1], F32, kind="ExternalInput").ap()

    from contextlib import ExitStack

    with tile.TileContext(nc) as tc:
        with ExitStack() as ctx:
            _emit(ctx, tc, aps, use_bias)
    nc.compile()
    return nc


_CACHE = {}


def _run(inputs, trace=False, **run_kwargs):
    x = np.ascontiguousarray(np.asarray(inputs["x"], dtype=np.float32))
    assert x.shape == (N_CORES, C, H, W_IMG), x.shape
    w1 = np.asarray(inputs["w1"], dtype=np.float32)
    w2 = np.asarray(inputs["w2"], dtype=np.float32)
    w3 = np.asarray(inputs["w3"], dtype=np.float32)
    w4 = np.asarray(inputs["w4"], dtype=np.float32)
    w5 = np.asarray(inputs["w5"], dtype=np.float32)
    b1 = np.asarray(inputs["b1"], dtype=np.float32)
    b2 = np.asarray(inputs["b2"], dtype=np.float32)
    b3 = np.asarray(inputs["b3"], dtype=np.float32)
    b4 = np.asarray(inputs["b4"], dtype=np.float32)
    b5 = np.asarray(inputs["b5"], dtype=np.float32)
    use_bias = bool(
        np.any(b1) or np.any(b2) or np.any(b3) or np.any(b4) or np.any(b5)
    )

    if use_bias not in _CACHE:
        _CACHE[use_bias] = _build(use_bias)
    nc = _CACHE[use_bias]

    BF = ml_dtypes.bfloat16
    E4 = ml_dtypes.float8_e4m3
    w12t = np.concatenate([w1.T, w2.T], axis=1)  # [512, 384]
    shared = {
        "w12q": np.ascontiguousarray((w12t * WSCALE).astype(E4)),
        "w3t": np.ascontiguousarray(w3.T.astype(BF)),
        "w4t": np.ascontiguousarray(w4.T.astype(BF)),
        "w5t": np.ascontiguousarray(w5.T.astype(BF)),
        "identb": np.eye(128, dtype=BF),
        "ident8": np.eye(128, dtype=E4),
    }
    if use_bias:
        shared["b12row"] = np.ascontiguousarray(
            (np.concatenate([b1, b2]) * WSCALE)[None, :], dtype=np.float32
        )
        shared["b3c"] = np.ascontiguousarray(b3[:, None])
        shared["b4c"] = np.ascontiguousarray(b4[:, None])
        shared["b5c"] = np.ascontiguousarray(b5[:, None])

    xr = x.reshape(N_CORES, C, HW)
    in_maps = [
        {
            "xq": np.ascontiguousarray(xr[b].astype(E4)),
            "xb": np.ascontiguousarray(xr[b].astype(BF)),
            **shared,
        }
        for b in range(N_CORES)
    ]
    res = run_bass_kernel_spmd(
        nc, in_maps, core_ids=list(range(N_CORES)), trace=trace, **run_kwargs
    )
    out = np.stack(
        [
            res.results[b]["out"].astype(np.float32).reshape(C, H, W_IMG)
            for b in range(N_CORES)
        ]
    )
    return out, res


def kernel(**inputs):
    out, _ = _run(inputs, trace=False)
    return out


# revision 41
# speedup vs baseline: 1.1645x; 1.1645x over previous
"""Trainium2 Bass kernel for the GCM (global context module) problem.

Computation per batch sample b (x_b = x[b] viewed as [C=512, HW=9216]):
    x1 = w1 @ x_b                      [128, HW]
    x2 = w2 @ x_b                      [256, HW]
    v  = softmax_all(x1 @ x2^T)        [128, 256]  (softmax over all 32768)
    n  = relu(v + w3 @ v)              [128, 256]
    z  = w4 @ n^T                      [256, 128]
    W  = w5 @ z                        [512, 128]  (collapses y/conv5: w5@(z@x1) == (w5@z)@x1)
    out = x_b + W @ x1                 [512, HW]

Sharding: data-parallel over batch, one sample per NeuronCore (8 cores).

v3 strategy: fp8 (e4m3) DoubleRow phase-1, bf16 residual/output.
  - x uploaded twice: fp8 plane (4.7 MB, feeds the phase-1 GEMMs) and
    bf16 plane (9.4 MB, feeds the residual add). Output bf16 (9.4 MB).
  - Phase 1: [x1T|x2T] = xT @ [w1T|w2T] as fp8 DoubleRow matmuls
    (K=256 per pass -> 2 passes instead of 4; 2 elem/cycle). Weights
    scaled by 64 so small weights stay in e4m3 normal range; the
    stored xts tiles are 64*[x1T|x2T] in fp8, v_psum = 4096*v.
    v accumulates via DoubleRow on subtile pairs (K=256 of hw).
  - Softmax: exp(v'/4096 - max'/4096) on ACT with fused row sums
    (accum_out); 1/S folded into the z copy, 1/64 into the W copy.
  - Phase 2: x1 reused via PE transposes of stored fp8 x1T tiles.
    Per 512-tile: oc0/1 through a 2-bank PSUM tile + one fused DVE
    add (residual from contiguous bf16 x block tile), oc2/3 add the
    residual on the PE (identity matmul) + one fused ACT copy.
    Output staged [128, 4, 1536] bf16 per block, last block DMA'd
    per-tile to shorten the tail.

Numerics: the softmax is a hard argmax (top-2 gap ~90 vs fp8-induced
v noise ~1.4 std) so fp8 cannot flip it; x_res carries ~5% fp8 error
on an absmax-0.047 branch (tolerance 0.108); bf16 passthrough of x
costs <= 0.016. Measured end-to-end ~3e-2 max abs diff vs the
0.108 gate.
"""

import numpy as np
import ml_dtypes

import concourse.bass as bass
import concourse.tile as tile
from concourse import bacc, mybir, bass_isa
from concourse.bass_utils import run_bass_kernel_spmd

F32 = mybir.dt.float32
BF16 = mybir.dt.bfloat16
FP8 = mybir.dt.float8e4
DR = mybir.MatmulPerfMode.DoubleRow
AX = mybir.AxisListType
AL = mybir.AluOpType
AF = mybir.ActivationFunctionType

N_CORES = 8
C = 512
H = W_IMG = 96
HW = H * W_IMG          # 9216
CK = C // 128           # 4 chunks of channels
NBLK = 6                # x blocks along hw
BLK = HW // NBLK        # 1536
NSUB = HW // 128        # 72 phase-1 subtiles
NPAIR = NSUB // 2       # 36 subtile pairs (DoubleRow v)
SUB_PER_BLK = BLK // 128
NT = HW // 512          # 18 phase-2 tiles
C4 = C // 4             # 128
C2 = C // 2             # 256
KM = C4 + C2            # 384 = concat(x1T, x2T) free size

WSCALE = 64.0           # fp8 weight scale (w12 * 64)
VSCALE = WSCALE * WSCALE  # v_psum = VSCALE * v


def _emit(ctx, tc, aps, use_bias):
    nc = tc.nc
    xq_d = aps["xq"]
    xb_d = aps["xb"]
    w12q_d = aps["w12q"]
    w3t_d = aps["w3t"]
    w4t_d = aps["w4t"]
    w5t_d = aps["w5t"]
    out_d = aps["out"]

    consts = ctx.enter_context(tc.tile_pool(name="consts", bufs=1))

    # Phase-1-critical DMAs go first on the sync queue (identb for PE
    # warmup, w12q + x block 0 for the first matmuls); the other consts
    # issue from the scalar queue so they don't delay them.
    identb = consts.tile([128, 128], BF16, tag="identb")
    nc.sync.dma_start(out=identb[:], in_=aps["identb"][:, :])

    # w12q: fp8, 64x, paired for DoubleRow: [128, 2, 384] per c-chunk pair
    w12 = []
    for q in range(2):
        t = consts.tile([128, 2, KM], FP8, tag=f"w12_{q}")
        for i in range(2):
            r0 = (2 * q + i) * 128
            nc.sync.dma_start(out=t[:, i, :], in_=w12q_d[r0 : r0 + 128, :])
        w12.append(t)
    ident8 = consts.tile([128, 128], FP8, tag="ident8")
    nc.scalar.dma_start(out=ident8[:], in_=aps["ident8"][:, :])
    w3t = consts.tile([128, 128], BF16, tag="w3t")
    nc.scalar.dma_start(out=w3t[:], in_=w3t_d[:, :])
    w4t = []
    for q in range(2):
        t = consts.tile([128, C2], BF16, tag=f"w4t_{q}")
        nc.scalar.dma_start(out=t[:], in_=w4t_d[q * 128 : (q + 1) * 128, :])
        w4t.append(t)
    w5t = []
    for q in range(2):
        t = consts.tile([128, C], BF16, tag=f"w5t_{q}")
        nc.scalar.dma_start(out=t[:], in_=w5t_d[q * 128 : (q + 1) * 128, :])
        w5t.append(t)

    bias_t = {}
    if use_bias:
        b12row_d = aps["b12row"]  # already scaled by WSCALE on host
        b3_d, b4_d, b5_d = aps["b3c"], aps["b4c"], aps["b5c"]
        brow1 = consts.tile([1, KM], F32, tag="brow1")
        nc.sync.dma_start(out=brow1[:], in_=b12row_d[:, :])
        brow = consts.tile([128, KM], F32, tag="brow")
        nc.gpsimd.partition_broadcast(brow[:], brow1[:])
        bias_t["brow"] = brow
        b3 = consts.tile([128, 1], F32, tag="b3")
        nc.sync.dma_start(out=b3[:], in_=b3_d[:, :])
        bias_t["b3"] = b3
        b4 = []
        for q in range(2):
            t = consts.tile([128, 1], F32, tag=f"b4_{q}")
            nc.sync.dma_start(out=t[:], in_=b4_d[q * 128 : (q + 1) * 128, :])
            b4.append(t)
        bias_t["b4"] = b4
        b5 = []
        for oc in range(CK):
            t = consts.tile([128, 1], F32, tag=f"b5_{oc}")
            nc.sync.dma_start(out=t[:], in_=b5_d[oc * 128 : (oc + 1) * 128, :])
            b5.append(t)
        bias_t["b5"] = b5

    # ---- x resident in SBUF ----
    # fp8 plane (phase-1 stationary operands) as one [128, 4, HW] tile,
    # the first 1536 hw cols split into [128,512] pieces for early
    # compute start. bf16 plane (residual) as one tile, 4 descriptors on
    # the otherwise-idle gpsimd DGE queue (it's only needed in phase 2).
    # Per-block tiles: dep tracking is per-tile for DMA writers, so one
    # big tile would stall the first matmul on the whole upload.
    xqpool = ctx.enter_context(tc.tile_pool(name="xq", bufs=1))
    xq = {}
    for b in range(NBLK):
        xq[b] = xqpool.tile([128, CK, BLK], FP8, tag=f"xq_{b}", name=f"xq_{b}")
    for p in range(BLK // 512):
        for c in range(CK):
            nc.sync.dma_start(
                out=xq[0][:, c, p * 512 : (p + 1) * 512],
                in_=xq_d[c * 128 : (c + 1) * 128, p * 512 : (p + 1) * 512],
            )
    for b in range(1, NBLK):
        for c in range(CK):
            nc.sync.dma_start(
                out=xq[b][:, c, :],
                in_=xq_d[c * 128 : (c + 1) * 128, b * BLK : (b + 1) * BLK],
            )
    # bf16 plane issued on the SAME sync queue AFTER all fp8 descriptors:
    # the DMA rings serve in issue order, so the phase-1-critical fp8
    # data is never starved by this 9.4 MB stream (it's only needed in
    # phase 2, ~40us later).
    xbpool = ctx.enter_context(tc.tile_pool(name="xb", bufs=1))
    xbt = xbpool.tile([128, CK, HW], BF16, tag="xb", name="xb")
    for b in range(NBLK):
        for c in range(CK):
            nc.sync.dma_start(
                out=xbt[:, c, b * BLK : (b + 1) * BLK],
                in_=xb_d[c * 128 : (c + 1) * 128, b * BLK : (b + 1) * BLK],
            )

    sm = ctx.enter_context(tc.tile_pool(name="sm", bufs=1))
    # persistent hw-major 64*[x1T|x2T] fp8 tiles, two subtiles per tile
    # (DoubleRow v contracts over 256 hw at a time)
    xtsp = ctx.enter_context(tc.tile_pool(name="xts", bufs=1))
    xts2 = [
        xtsp.tile([128, 2, KM], FP8, tag=f"xts_{j}", name=f"xts_{j}")
        for j in range(NPAIR)
    ]
    # k-major 64*x1 fp8 tiles, two phase-2 tiles per SBUF tile
    x1pool = ctx.enter_context(tc.tile_pool(name="x1", bufs=1))
    x1sb = [
        x1pool.tile([128, 1024], FP8, tag=f"x1_{k}", name=f"x1_{k}")
        for k in range(NT // 2)
    ]

    e = sm.tile([128, C2], BF16, tag="e")
    s1 = sm.tile([128, 1], F32, tag="s1")

    # ---- phase 1: x12T tiles + v accumulation; x1 transposes; softmax ----
    with (
        tc.tile_pool(name="psC", bufs=1, space="PSUM") as psC,
        tc.tile_pool(name="psA", bufs=3, space="PSUM") as psA,
        tc.tile_pool(name="vps", bufs=1, space="PSUM") as vps,
    ):
        v_ps = vps.tile([128, C2], F32, tag="v")

        # Warm the PE HAM clock-gate during the initial x-DMA window so
        # real phase-1 matmuls start at full clock. Targets the v bank
        # (v's first real matmul has start=True, which resets it).
        # 48 matmuls ~= 5us of sustained activity, ending right as the
        # first x block lands - no idle gap for the HAM MID window.
        for _ in range(48):
            nc.tensor.matmul(
                v_ps[:, 0:128], identb[:], identb[:], start=True, stop=True
            )

        def emit_v(j):
            nc.tensor.matmul(
                v_ps[:],
                xts2[j][:, :, 0:C4],
                xts2[j][:, :, C4:KM],
                start=(j == 0),
                stop=(j == NPAIR - 1),
                perf_mode=DR,
            )

        def emit_tr_group(kk):
            # 8 transposes (4 xts2 pairs -> one k-major [128,1024] x1
            # tile) into one PSUM bank, then one fused copy to SBUF.
            # fp8 PE transposes must write with element step 2 (HW
            # quirk); the copy reads the same strided view.
            pc = psC.tile([128, 2048], FP8, tag="x1T", name=f"x1T_{kk}")
            for j in range(8):
                s = 8 * kk + j
                nc.tensor.transpose(
                    pc[:, j * 256 : (j + 1) * 256 : 2],
                    xts2[s // 2][:, s % 2, 0:C4],
                    ident8[:],
                )
            if kk % 2 == 0:
                nc.scalar.copy(x1sb[kk][:], pc[:, 0:2048:2])
            else:
                nc.vector.tensor_copy(x1sb[kk][:], pc[:, 0:2048:2])

        # psA tiles hold a PAIR of subtiles ([128,2,512] = 2 banks, 384
        # used per bank) so each pair drains with ONE fused copy,
        # alternating ACT/DVE (per-op fixed cost dominates these engines).
        # Every 4th pair, the finished x1T tiles are transposed back to
        # k-major in the PE stream (phase-1 PE has slack vs the copies).
        SKEW = 2  # pairs
        pend = []
        for j in range(NPAIR):
            b, poff = divmod(j, SUB_PER_BLK // 2)
            ps = psA.tile([128, 2, 512], F32, tag="xts")
            for i in range(2):
                off = (2 * poff + i) * 128
                for q in range(2):
                    nc.tensor.matmul(
                        ps[:, i, 0:KM],
                        xq[b][:, 2 * q : 2 * q + 2, off : off + 128],
                        w12[q][:],
                        start=(q == 0),
                        stop=(q == 1),
                        perf_mode=DR,
                    )
            if use_bias:
                for i in range(2):
                    nc.vector.tensor_tensor(
                        xts2[j][:, i, :], ps[:, i, 0:KM],
                        bias_t["brow"][:], op=AL.add,
                    )
            elif j % 2 == 0:
                nc.scalar.copy(xts2[j][:], ps[:, :, 0:KM])
            else:
                nc.vector.tensor_copy(xts2[j][:], ps[:, :, 0:KM])
            pend.append(j)
            if len(pend) > SKEW:
                emit_v(pend.pop(0))
            if j % 4 == 3:
                emit_tr_group(j // 4)
        while pend:
            emit_v(pend.pop(0))

        # ---- softmax over all 32768 entries of v (unnormalized exp) ----
        # v_ps holds VSCALE*v; fold 1/VSCALE into the exp scale/bias.
        m1 = sm.tile([128, 1], F32, tag="m1")
        nc.vector.tensor_reduce(m1[:], v_ps[:], axis=AX.X, op=AL.max)
        mall = sm.tile([128, 1], F32, tag="mall")
        nc.gpsimd.partition_all_reduce(mall[:], m1[:], 128, bass_isa.ReduceOp.max)
        negm = sm.tile([128, 1], F32, tag="negm")
        nc.vector.tensor_scalar_mul(negm[:], mall[:], -1.0 / VSCALE)
        nc.scalar.activation(
            e[:], v_ps[:], AF.Exp, bias=negm[:], scale=1.0 / VSCALE,
            accum_out=s1[:],
        )

    sall = sm.tile([128, 1], F32, tag="sall")
    nc.gpsimd.partition_all_reduce(sall[:], s1[:], 128, bass_isa.ReduceOp.add)
    sinv = sm.tile([128, 1], F32, tag="sinv")
    nc.vector.reciprocal(sinv[:], sall[:])

    wt = sm.tile([128, C], BF16, tag="wt")
    if True:
        with tc.tile_pool(name="psB", bufs=1, space="PSUM") as psB:
            # ---- small chain: conv3+relu, n^T, z (1/S folded), W^T ----
            if use_bias:
                en = sm.tile([128, C2], BF16, tag="en")
                nc.vector.tensor_scalar_mul(en[:], e[:], sinv[:])
                esrc = en
            else:
                esrc = e
            # The PE idles during the softmax/chain serial section; ~3.4us
            # of idle re-throttles the HAM clock gate to 1.2GHz and the
            # next ~30 matmuls run at half speed. Dummy matmuls into
            # not-yet-live PSUM banks keep it warm (each real first
            # matmul below has start=True, which resets the bank).
            ps3 = psB.tile([128, C2], F32, tag="ps3")
            pz = psB.tile([128, C2], F32, tag="pz")
            pW = psB.tile([128, C], F32, tag="pW")

            def keep_warm(ps, n):
                for _ in range(n):
                    nc.tensor.matmul(
                        ps, identb[:], identb[:], start=True, stop=True
                    )

            keep_warm(ps3[:, 0:128], 20)
            # n = relu(e + w3@e): the +e rides on the PE as an identity
            # matmul into the same PSUM bank, relu is a single ACT op.
            nc.tensor.matmul(ps3[:], w3t[:], esrc[:], start=True, stop=False)
            nc.tensor.matmul(ps3[:], identb[:], esrc[:], start=False, stop=True)
            nsb = sm.tile([128, C2], BF16, tag="nsb")
            if use_bias:
                nc.scalar.activation(
                    nsb[:], ps3[:], AF.Relu, bias=bias_t["b3"][:]
                )
            else:
                nc.scalar.activation(nsb[:], ps3[:], AF.Relu)

            keep_warm(pz[:, 0:128], 10)
            pT = psB.tile([128, C2], BF16, tag="pT")
            for q in range(2):
                nc.tensor.transpose(
                    pT[:, q * 128 : (q + 1) * 128],
                    nsb[:, q * 128 : (q + 1) * 128],
                    identb[:],
                )
            ntt = sm.tile([128, C2], BF16, tag="ntt")
            nc.scalar.copy(ntt[:], pT[:])
            nts = [ntt[:, 0:128], ntt[:, 128:256]]

            keep_warm(pW[:, 0:128], 10)
            for mc in range(2):
                pzs = pz[:, mc * 128 : (mc + 1) * 128]
                for q in range(2):
                    nc.tensor.matmul(
                        pzs,
                        w4t[q][:, mc * 128 : (mc + 1) * 128],
                        nts[q],
                        start=(q == 0),
                        stop=(q == 1),
                    )
            zt = sm.tile([128, C2], BF16, tag="zt")
            zs = [zt[:, 0:128], zt[:, 128:256]]
            if use_bias:
                for mc in range(2):
                    nc.scalar.add(
                        zs[mc], pz[:, mc * 128 : (mc + 1) * 128],
                        bias_t["b4"][mc][:],
                    )
            else:
                # fold softmax 1/S here: W = w5 @ (z/S), one fused op
                nc.vector.tensor_scalar_mul(zt[:], pz[:], sinv[:])

            for mc in range(2):
                nc.tensor.matmul(
                    pW[:], zs[mc], w5t[mc][:], start=(mc == 0), stop=(mc == 1)
                )
            # 1/WSCALE cancels the 64x in the fp8 x1 tiles
            nc.scalar.activation(wt[:], pW[:], AF.Copy, scale=1.0 / WSCALE)

        # ---- phase 2: x_res = W @ x1, residual, staged DMA out ----
        # oc0/1: 2-bank PSUM tile + one fused DVE add (bf16 x residual).
        # oc2/3: residual via PE identity matmul + one fused ACT copy.
        with (
            tc.tile_pool(name="psD", bufs=2, space="PSUM") as psD,
            tc.tile_pool(name="psE", bufs=2, space="PSUM") as psE,
            tc.tile_pool(name="outp", bufs=4) as outp,
        ):
            stage = None
            for t in range(NT):
                g, ti = divmod(t, 3)
                off = ti * 512  # offset within block g (BLK==1536==3*512)
                hw0 = t * 512
                k, half = divmod(t, 2)
                x1v = x1sb[k][:, half * 512 : (half + 1) * 512]
                if ti == 0:
                    stage = outp.tile(
                        [128, CK, BLK], BF16, tag="st", name=f"st_{g}"
                    )
                pr = psD.tile([128, 2, 512], F32, tag="pr")
                for oc in range(2):
                    nc.tensor.matmul(
                        pr[:, oc, :],
                        wt[:, oc * 128 : (oc + 1) * 128],
                        x1v,
                        start=True,
                        stop=True,
                    )
                pe = psE.tile([128, 2, 512], F32, tag="pe")
                for oc in range(2, 4):
                    pes = pe[:, oc - 2, :]
                    nc.tensor.matmul(
                        pes, wt[:, oc * 128 : (oc + 1) * 128], x1v,
                        start=True, stop=False,
                    )
                    nc.tensor.matmul(
                        pes, identb[:], xbt[:, oc, hw0 : hw0 + 512],
                        start=False, stop=True,
                    )
                if use_bias:
                    for oc in range(2):
                        nc.vector.scalar_tensor_tensor(
                            stage[:, oc, off : off + 512],
                            pr[:, oc, :],
                            bias_t["b5"][oc][:],
                            xbt[:, oc, hw0 : hw0 + 512],
                            op0=AL.add,
                            op1=AL.add,
                        )
                    for oc in range(2, 4):
                        nc.scalar.add(
                            stage[:, oc, off : off + 512],
                            pe[:, oc - 2, :],
                            bias_t["b5"][oc][:],
                        )
                else:
                    nc.vector.tensor_tensor(
                        stage[:, 0:2, off : off + 512],
                        pr[:],
                        xbt[:, 0:2, hw0 : hw0 + 512],
                        op=AL.add,
                    )
                    nc.scalar.copy(stage[:, 2:4, off : off + 512], pe[:])
                # one descriptor per block via a [128, 4, win] dram view
                # (dim0 = partition, dim1 = oc chunk); last block DMA'd
                # per tile to shorten the pipeline-drain tail.
                outv = out_d.rearrange("(o p) w -> p o w", o=CK)
                if g == NBLK - 1:
                    hw1 = g * BLK + off
                    nc.gpsimd.dma_start(
                        out=outv[:, :, hw1 : hw1 + 512],
                        in_=stage[:, :, off : off + 512],
                    )
                elif ti == 2:
                    nc.gpsimd.dma_start(
                        out=outv[:, :, g * BLK : (g + 1) * BLK],
                        in_=stage[:],
                    )


def _build(use_bias):
    nc = bacc.Bacc("TRN2", target_bir_lowering=False, debug=False, num_devices=N_CORES)
    aps = {
        "xq": nc.dram_tensor("xq", [C, HW], FP8, kind="ExternalInput").ap(),
        "xb": nc.dram_tensor("xb", [C, HW], BF16, kind="ExternalInput").ap(),
        "w12q": nc.dram_tensor("w12q", [C, KM], FP8, kind="ExternalInput").ap(),
        "w3t": nc.dram_tensor("w3t", [C4, C4], BF16, kind="ExternalInput").ap(),
        "w4t": nc.dram_tensor("w4t", [C2, C2], BF16, kind="ExternalInput").ap(),
        "w5t": nc.dram_tensor("w5t", [C2, C], BF16, kind="ExternalInput").ap(),
        "identb": nc.dram_tensor(
            "identb", [128, 128], BF16, kind="ExternalInput"
        ).ap(),
        "ident8": nc.dram_tensor(
            "ident8", [128, 128], FP8, kind="ExternalInput"
        ).ap(),
        "out": nc.dram_tensor("out", [C, HW], BF16, kind="ExternalOutput").ap(),
    }
    if use_bias:
        aps["b12row"] = nc.dram_tensor(
            "b12row", [1, KM], F32, kind="ExternalInput"
        ).ap()
        aps["b3c"] = nc.dram_tensor("b3c", [C4, 1], F32, kind="ExternalInput").ap()
        aps["b4c"] = nc.dram_tensor("b4c", [C2, 1], F32, kind="ExternalInput").ap()
        aps["b5c"] = nc.dram_tensor("b5c", [C, 1], F32, kind="ExternalInput").ap()

    from contextlib import ExitStack

    with tile.TileContext(nc) as tc:
        with ExitStack() as ctx:
            _emit(ctx, tc, aps, use_bias)
    nc.compile()
    return nc


_CACHE = {}


def _run(inputs, trace=False, **run_kwargs):
    x = np.ascontiguousarray(np.asarray(inputs["x"], dtype=np.float32))
    assert x.shape == (N_CORES, C, H, W_IMG), x.shape
    w1 = np.asarray(inputs["w1"], dtype=np.float32)
    w2 = np.asarray(inputs["w2"], dtype=np.float32)
    w3 = np.asarray(inputs["w3"], dtype=np.float32)
    w4 = np.asarray(inputs["w4"], dtype=np.float32)
    w5 = np.asarray(inputs["w5"], dtype=np.float32)
    b1 = np.asarray(inputs["b1"], dtype=np.float32)
    b2 = np.asarray(inputs["b2"], dtype=np.float32)
    b3 = np.asarray(inputs["b3"], dtype=np.float32)
    b4 = np.asarray(inputs["b4"], dtype=np.float32)
    b5 = np.asarray(inputs["b5"], dtype=np.float32)
    use_bias = bool(
        np.any(b1) or np.any(b2) or np.any(b3) or np.any(b4) or np.any(b5)
    )

    if use_bias not in _CACHE:
        _CACHE[use_bias] = _build(use_bias)
    nc = _CACHE[use_bias]

    BF = ml_dtypes.bfloat16
    E4 = ml_dtypes.float8_e4m3
    w12t = np.concatenate([w1.T, w2.T], axis=1)  # [512, 384]
    shared = {
        "w12q": np.ascontiguousarray((w12t * WSCALE).astype(E4)),
        "w3t": np.ascontiguousarray(w3.T.astype(BF)),
        "w4t": np.ascontiguousarray(w4.T.astype(BF)),
        "w5t": np.ascontiguousarray(w5.T.astype(BF)),
        "identb": np.eye(128, dtype=BF),
        "ident8": np.eye(128, dtype=E4),
    }
    if use_bias:
        shared["b12row"] = np.ascontiguousarray(
            (np.concatenate([b1, b2]) * WSCALE)[None, :], dtype=np.float32
        )
        shared["b3c"] = np.ascontiguousarray(b3[:, None])
        shared["b4c"] = np.ascontiguousarray(b4[:, None])
        shared["b5c"] = np.ascontiguousarray(b5[:, None])

    xr = x.reshape(N_CORES, C, HW)
    in_maps = [
        {
            "xq": np.ascontiguousarray(xr[b].astype(E4)),
            "xb": np.ascontiguousarray(xr[b].astype(BF)),
            **shared,
        }
        for b in range(N_CORES)
    ]
    res = run_bass_kernel_spmd(
        nc, in_maps, core_ids=list(range(N_CORES)), trace=trace, **run_kwargs
    )
    out = np.stack(
        [
            res.results[b]["out"].astype(np.float32).reshape(C, H, W_IMG)
            for b in range(N_CORES)
        ]
    )
    return out, res


def kernel(**inputs):
    out, _ = _run(inputs, trace=False)
    return out


# revision 43
# speedup vs baseline: 1.2315x; 1.0576x over previous
"""Trainium2 Bass kernel for the GCM (global context module) problem.

Computation per batch sample b (x_b = x[b] viewed as [C=512, HW=9216]):
    x1 = w1 @ x_b                      [128, HW]
    x2 = w2 @ x_b                      [256, HW]
    v  = softmax_all(x1 @ x2^T)        [128, 256]  (softmax over all 32768)
    n  = relu(v + w3 @ v)              [128, 256]
    z  = w4 @ n^T                      [256, 128]
    W  = w5 @ z                        [512, 128]  (collapses y/conv5: w5@(z@x1) == (w5@z)@x1)
    out = x_b + W @ x1                 [512, HW]

Sharding: data-parallel over batch, one sample per NeuronCore (8 cores).

v3 strategy: fp8 (e4m3) DoubleRow phase-1, bf16 residual/output.
  - x uploaded twice: fp8 plane (4.7 MB, feeds the phase-1 GEMMs) and
    bf16 plane (9.4 MB, feeds the residual add). Output bf16 (9.4 MB).
  - Phase 1: [x1T|x2T] = xT @ [w1T|w2T] as fp8 DoubleRow matmuls
    (K=256 per pass -> 2 passes instead of 4; 2 elem/cycle). Weights
    scaled by 64 so small weights stay in e4m3 normal range; the
    stored xts tiles are 64*[x1T|x2T] in fp8, v_psum = 4096*v.
    v accumulates via DoubleRow on subtile pairs (K=256 of hw).
  - Softmax: exp(v'/4096 - max'/4096) on ACT with fused row sums
    (accum_out); 1/S folded into the z copy, 1/64 into the W copy.
  - Phase 2: x1 reused via PE transposes of stored fp8 x1T tiles.
    Per 512-tile: oc0/1 through a 2-bank PSUM tile + one fused DVE
    add (residual from contiguous bf16 x block tile), oc2/3 add the
    residual on the PE (identity matmul) + one fused ACT copy.
    Output staged [128, 4, 1536] bf16 per block, last block DMA'd
    per-tile to shorten the tail.

Numerics: the softmax is a hard argmax (top-2 gap ~90 vs fp8-induced
v noise ~1.4 std) so fp8 cannot flip it; x_res carries ~5% fp8 error
on an absmax-0.047 branch (tolerance 0.108); bf16 passthrough of x
costs <= 0.016. Measured end-to-end ~3e-2 max abs diff vs the
0.108 gate.
"""

import numpy as np
import ml_dtypes

import concourse.bass as bass
import concourse.tile as tile
from concourse import bacc, mybir, bass_isa
from concourse.bass_utils import run_bass_kernel_spmd

F32 = mybir.dt.float32
BF16 = mybir.dt.bfloat16
FP8 = mybir.dt.float8e4
DR = mybir.MatmulPerfMode.DoubleRow
AX = mybir.AxisListType
AL = mybir.AluOpType
AF = mybir.ActivationFunctionType

N_CORES = 8
C = 512
H = W_IMG = 96
HW = H * W_IMG          # 9216
CK = C // 128           # 4 chunks of channels
NBLK = 6                # x blocks along hw
BLK = HW // NBLK        # 1536
NSUB = HW // 128        # 72 phase-1 subtiles
NPAIR = NSUB // 2       # 36 subtile pairs (DoubleRow v)
SUB_PER_BLK = BLK // 128
NT = HW // 512          # 18 phase-2 tiles
C4 = C // 4             # 128
C2 = C // 2             # 256
KM = C4 + C2            # 384 = concat(x1T, x2T) free size

WSCALE = 64.0           # fp8 weight scale (w12 * 64)
VSCALE = WSCALE * WSCALE  # v_psum = VSCALE * v


def _emit(ctx, tc, aps, use_bias):
    nc = tc.nc
    xq_d = aps["xq"]
    xb_d = aps["xb"]
    w12q_d = aps["w12q"]
    w3t_d = aps["w3t"]
    w4t_d = aps["w4t"]
    w5t_d = aps["w5t"]
    out_d = aps["out"]

    consts = ctx.enter_context(tc.tile_pool(name="consts", bufs=1))

    # The x fp8 plane owns the sync queue (it paces early phase-1); all
    # consts issue in parallel from the scalar queue (idle at startup).
    identb = consts.tile([128, 128], BF16, tag="identb")
    nc.scalar.dma_start(out=identb[:], in_=aps["identb"][:, :])

    # w12q: fp8, 64x, paired for DoubleRow: [128, 2, 384] per c-chunk pair
    w12 = []
    for q in range(2):
        t = consts.tile([128, 2, KM], FP8, tag=f"w12_{q}")
        for i in range(2):
            r0 = (2 * q + i) * 128
            nc.scalar.dma_start(out=t[:, i, :], in_=w12q_d[r0 : r0 + 128, :])
        w12.append(t)
    ident8 = consts.tile([128, 128], FP8, tag="ident8")
    nc.scalar.dma_start(out=ident8[:], in_=aps["ident8"][:, :])
    w3t = consts.tile([128, 128], BF16, tag="w3t")
    nc.scalar.dma_start(out=w3t[:], in_=w3t_d[:, :])
    w4t = []
    for q in range(2):
        t = consts.tile([128, C2], BF16, tag=f"w4t_{q}")
        nc.scalar.dma_start(out=t[:], in_=w4t_d[q * 128 : (q + 1) * 128, :])
        w4t.append(t)
    w5t = []
    for q in range(2):
        t = consts.tile([128, C], BF16, tag=f"w5t_{q}")
        nc.scalar.dma_start(out=t[:], in_=w5t_d[q * 128 : (q + 1) * 128, :])
        w5t.append(t)

    bias_t = {}
    if use_bias:
        b12row_d = aps["b12row"]  # already scaled by WSCALE on host
        b3_d, b4_d, b5_d = aps["b3c"], aps["b4c"], aps["b5c"]
        brow1 = consts.tile([1, KM], F32, tag="brow1")
        nc.scalar.dma_start(out=brow1[:], in_=b12row_d[:, :])
        brow = consts.tile([128, KM], F32, tag="brow")
        nc.gpsimd.partition_broadcast(brow[:], brow1[:])
        bias_t["brow"] = brow
        b3 = consts.tile([128, 1], F32, tag="b3")
        nc.scalar.dma_start(out=b3[:], in_=b3_d[:, :])
        bias_t["b3"] = b3
        b4 = []
        for q in range(2):
            t = consts.tile([128, 1], F32, tag=f"b4_{q}")
            nc.scalar.dma_start(out=t[:], in_=b4_d[q * 128 : (q + 1) * 128, :])
            b4.append(t)
        bias_t["b4"] = b4
        b5 = []
        for oc in range(CK):
            t = consts.tile([128, 1], F32, tag=f"b5_{oc}")
            nc.scalar.dma_start(out=t[:], in_=b5_d[oc * 128 : (oc + 1) * 128, :])
            b5.append(t)
        bias_t["b5"] = b5

    # ---- x resident in SBUF ----
    # fp8 plane (phase-1 stationary operands) as one [128, 4, HW] tile,
    # the first 1536 hw cols split into [128,512] pieces for early
    # compute start. bf16 plane (residual) as one tile, 4 descriptors on
    # the otherwise-idle gpsimd DGE queue (it's only needed in phase 2).
    # Per-block tiles: dep tracking is per-tile for DMA writers, so one
    # big tile would stall the first matmul on the whole upload.
    xqpool = ctx.enter_context(tc.tile_pool(name="xq", bufs=1))
    xq = {}
    for b in range(NBLK):
        xq[b] = xqpool.tile([128, CK, BLK], FP8, tag=f"xq_{b}", name=f"xq_{b}")
    for p in range(BLK // 512):
        for c in range(CK):
            nc.sync.dma_start(
                out=xq[0][:, c, p * 512 : (p + 1) * 512],
                in_=xq_d[c * 128 : (c + 1) * 128, p * 512 : (p + 1) * 512],
            )
    for b in range(1, NBLK):
        for c in range(CK):
            nc.sync.dma_start(
                out=xq[b][:, c, :],
                in_=xq_d[c * 128 : (c + 1) * 128, b * BLK : (b + 1) * BLK],
            )
    # bf16 plane issued on the SAME sync queue AFTER all fp8 descriptors:
    # the DMA rings serve in issue order, so the phase-1-critical fp8
    # data is never starved by this 9.4 MB stream (it's only needed in
    # phase 2, ~40us later).
    xbpool = ctx.enter_context(tc.tile_pool(name="xb", bufs=1))
    xbt = xbpool.tile([128, CK, HW], BF16, tag="xb", name="xb")
    for b in range(NBLK):
        for c in range(CK):
            nc.sync.dma_start(
                out=xbt[:, c, b * BLK : (b + 1) * BLK],
                in_=xb_d[c * 128 : (c + 1) * 128, b * BLK : (b + 1) * BLK],
            )

    sm = ctx.enter_context(tc.tile_pool(name="sm", bufs=1))
    # persistent hw-major 64*[x1T|x2T] fp8 tiles, two subtiles per tile
    # (DoubleRow v contracts over 256 hw at a time)
    xtsp = ctx.enter_context(tc.tile_pool(name="xts", bufs=1))
    xts2 = [
        xtsp.tile([128, 2, KM], FP8, tag=f"xts_{j}", name=f"xts_{j}")
        for j in range(NPAIR)
    ]
    # k-major 64*x1 fp8 tiles, two phase-2 tiles per SBUF tile
    x1pool = ctx.enter_context(tc.tile_pool(name="x1", bufs=1))
    x1sb = [
        x1pool.tile([128, 1024], FP8, tag=f"x1_{k}", name=f"x1_{k}")
        for k in range(NT // 2)
    ]

    e = sm.tile([128, C2], BF16, tag="e")
    s1 = sm.tile([128, 1], F32, tag="s1")

    # ---- phase 1: x12T tiles + v accumulation; x1 transposes; softmax ----
    with (
        tc.tile_pool(name="psC", bufs=1, space="PSUM") as psC,
        tc.tile_pool(name="psA", bufs=3, space="PSUM") as psA,
        tc.tile_pool(name="vps", bufs=1, space="PSUM") as vps,
    ):
        v_ps = vps.tile([128, C2], F32, tag="v")

        # Warm the PE HAM clock-gate during the initial x-DMA window so
        # real phase-1 matmuls start at full clock. Targets the v bank
        # (v's first real matmul has start=True, which resets it).
        # 48 matmuls ~= 5us of sustained activity, ending right as the
        # first x block lands - no idle gap for the HAM MID window.
        for _ in range(40):
            nc.tensor.matmul(
                v_ps[:, 0:128], identb[:], identb[:], start=True, stop=True
            )

        def emit_v(j):
            nc.tensor.matmul(
                v_ps[:],
                xts2[j][:, :, 0:C4],
                xts2[j][:, :, C4:KM],
                start=(j == 0),
                stop=(j == NPAIR - 1),
                perf_mode=DR,
            )

        def emit_tr_group(kk):
            # 8 transposes (4 xts2 pairs -> one k-major [128,1024] x1
            # tile) into one PSUM bank, then one fused copy to SBUF.
            # fp8 PE transposes must write with element step 2 (HW
            # quirk); the copy reads the same strided view.
            pc = psC.tile([128, 2048], FP8, tag="x1T", name=f"x1T_{kk}")
            for j in range(8):
                s = 8 * kk + j
                nc.tensor.transpose(
                    pc[:, j * 256 : (j + 1) * 256 : 2],
                    xts2[s // 2][:, s % 2, 0:C4],
                    ident8[:],
                )
            if kk % 2 == 0:
                nc.scalar.copy(x1sb[kk][:], pc[:, 0:2048:2])
            else:
                nc.vector.tensor_copy(x1sb[kk][:], pc[:, 0:2048:2])

        # psA tiles hold a PAIR of subtiles ([128,2,512] = 2 banks, 384
        # used per bank) so each pair drains with ONE fused copy,
        # alternating ACT/DVE (per-op fixed cost dominates these engines).
        # Every 4th pair, the finished x1T tiles are transposed back to
        # k-major in the PE stream (phase-1 PE has slack vs the copies).
        SKEW = 2  # pairs
        pend = []
        for j in range(NPAIR):
            b, poff = divmod(j, SUB_PER_BLK // 2)
            ps = psA.tile([128, 2, 512], F32, tag="xts")
            for i in range(2):
                off = (2 * poff + i) * 128
                for q in range(2):
                    nc.tensor.matmul(
                        ps[:, i, 0:KM],
                        xq[b][:, 2 * q : 2 * q + 2, off : off + 128],
                        w12[q][:],
                        start=(q == 0),
                        stop=(q == 1),
                        perf_mode=DR,
                    )
            if use_bias:
                for i in range(2):
                    nc.vector.tensor_tensor(
                        xts2[j][:, i, :], ps[:, i, 0:KM],
                        bias_t["brow"][:], op=AL.add,
                    )
            elif j % 2 == 0:
                nc.scalar.copy(xts2[j][:], ps[:, :, 0:KM])
            else:
                nc.vector.tensor_copy(xts2[j][:], ps[:, :, 0:KM])
            pend.append(j)
            if len(pend) > SKEW:
                emit_v(pend.pop(0))
            if j % 4 == 3:
                emit_tr_group(j // 4)
        while pend:
            emit_v(pend.pop(0))

        # ---- softmax over all 32768 entries of v (unnormalized exp) ----
        # v_ps holds VSCALE*v; fold 1/VSCALE into the exp scale/bias.
        m1 = sm.tile([128, 1], F32, tag="m1")
        nc.vector.tensor_reduce(m1[:], v_ps[:], axis=AX.X, op=AL.max)
        mall = sm.tile([128, 1], F32, tag="mall")
        nc.gpsimd.partition_all_reduce(mall[:], m1[:], 128, bass_isa.ReduceOp.max)
        negm = sm.tile([128, 1], F32, tag="negm")
        nc.vector.tensor_scalar_mul(negm[:], mall[:], -1.0 / VSCALE)
        nc.scalar.activation(
            e[:], v_ps[:], AF.Exp, bias=negm[:], scale=1.0 / VSCALE,
            accum_out=s1[:],
        )

    sall = sm.tile([128, 1], F32, tag="sall")
    nc.gpsimd.partition_all_reduce(sall[:], s1[:], 128, bass_isa.ReduceOp.add)
    sinv = sm.tile([128, 1], F32, tag="sinv")
    nc.vector.reciprocal(sinv[:], sall[:])

    wt = sm.tile([128, C], BF16, tag="wt")
    if True:
        with tc.tile_pool(name="psB", bufs=1, space="PSUM") as psB:
            # ---- small chain: conv3+relu, n^T, z (1/S folded), W^T ----
            if use_bias:
                en = sm.tile([128, C2], BF16, tag="en")
                nc.vector.tensor_scalar_mul(en[:], e[:], sinv[:])
                esrc = en
            else:
                esrc = e
            # The PE idles during the softmax/chain serial section; ~3.4us
            # of idle re-throttles the HAM clock gate to 1.2GHz and the
            # next ~30 matmuls run at half speed. Dummy matmuls into
            # not-yet-live PSUM banks keep it warm (each real first
            # matmul below has start=True, which resets the bank).
            ps3 = psB.tile([128, C2], F32, tag="ps3")
            pz = psB.tile([128, C2], F32, tag="pz")
            pW = psB.tile([128, C], F32, tag="pW")

            def keep_warm(ps, n):
                for _ in range(n):
                    nc.tensor.matmul(
                        ps, identb[:], identb[:], start=True, stop=True
                    )

            keep_warm(ps3[:, 0:128], 20)
            # n = relu(e + w3@e): the +e rides on the PE as an identity
            # matmul into the same PSUM bank, relu is a single ACT op.
            nc.tensor.matmul(ps3[:], w3t[:], esrc[:], start=True, stop=False)
            nc.tensor.matmul(ps3[:], identb[:], esrc[:], start=False, stop=True)
            nsb = sm.tile([128, C2], BF16, tag="nsb")
            if use_bias:
                nc.scalar.activation(
                    nsb[:], ps3[:], AF.Relu, bias=bias_t["b3"][:]
                )
            else:
                nc.scalar.activation(nsb[:], ps3[:], AF.Relu)

            keep_warm(pz[:, 0:128], 10)
            pT = psB.tile([128, C2], BF16, tag="pT")
            for q in range(2):
                nc.tensor.transpose(
                    pT[:, q * 128 : (q + 1) * 128],
                    nsb[:, q * 128 : (q + 1) * 128],
                    identb[:],
                )
            ntt = sm.tile([128, C2], BF16, tag="ntt")
            nc.scalar.copy(ntt[:], pT[:])
            nts = [ntt[:, 0:128], ntt[:, 128:256]]

            keep_warm(pW[:, 0:128], 10)
            for mc in range(2):
                pzs = pz[:, mc * 128 : (mc + 1) * 128]
                for q in range(2):
                    nc.tensor.matmul(
                        pzs,
                        w4t[q][:, mc * 128 : (mc + 1) * 128],
                        nts[q],
                        start=(q == 0),
                        stop=(q == 1),
                    )
            zt = sm.tile([128, C2], BF16, tag="zt")
            zs = [zt[:, 0:128], zt[:, 128:256]]
            if use_bias:
                for mc in range(2):
                    nc.scalar.add(
                        zs[mc], pz[:, mc * 128 : (mc + 1) * 128],
                        bias_t["b4"][mc][:],
                    )
            else:
                # fold softmax 1/S here: W = w5 @ (z/S), one fused op
                nc.vector.tensor_scalar_mul(zt[:], pz[:], sinv[:])

            for mc in range(2):
                nc.tensor.matmul(
                    pW[:], zs[mc], w5t[mc][:], start=(mc == 0), stop=(mc == 1)
                )
            # 1/WSCALE cancels the 64x in the fp8 x1 tiles
            nc.scalar.activation(wt[:], pW[:], AF.Copy, scale=1.0 / WSCALE)

        # ---- phase 2: x_res = W @ x1, residual, staged DMA out ----
        # oc0/1: 2-bank PSUM tile + one fused DVE add (bf16 x residual).
        # oc2/3: residual via PE identity matmul + one fused ACT copy.
        with (
            tc.tile_pool(name="psD", bufs=2, space="PSUM") as psD,
            tc.tile_pool(name="psE", bufs=2, space="PSUM") as psE,
            tc.tile_pool(name="outp", bufs=4) as outp,
        ):
            stage = None
            for t in range(NT):
                g, ti = divmod(t, 3)
                off = ti * 512  # offset within block g (BLK==1536==3*512)
                hw0 = t * 512
                k, half = divmod(t, 2)
                x1v = x1sb[k][:, half * 512 : (half + 1) * 512]
                if ti == 0:
                    stage = outp.tile(
                        [128, CK, BLK], BF16, tag="st", name=f"st_{g}"
                    )
                pr = psD.tile([128, 2, 512], F32, tag="pr")
                for oc in range(2):
                    nc.tensor.matmul(
                        pr[:, oc, :],
                        wt[:, oc * 128 : (oc + 1) * 128],
                        x1v,
                        start=True,
                        stop=True,
                    )
                pe = psE.tile([128, 2, 512], F32, tag="pe")
                for oc in range(2, 4):
                    pes = pe[:, oc - 2, :]
                    nc.tensor.matmul(
                        pes, wt[:, oc * 128 : (oc + 1) * 128], x1v,
                        start=True, stop=False,
                    )
                    nc.tensor.matmul(
                        pes, identb[:], xbt[:, oc, hw0 : hw0 + 512],
                        start=False, stop=True,
                    )
                if use_bias:
                    for oc in range(2):
                        nc.vector.scalar_tensor_tensor(
                            stage[:, oc, off : off + 512],
                            pr[:, oc, :],
                            bias_t["b5"][oc][:],
                            xbt[:, oc, hw0 : hw0 + 512],
                            op0=AL.add,
                            op1=AL.add,
                        )
                    for oc in range(2, 4):
                        nc.scalar.add(
                            stage[:, oc, off : off + 512],
                            pe[:, oc - 2, :],
                            bias_t["b5"][oc][:],
                        )
                else:
                    nc.vector.tensor_tensor(
                        stage[:, 0:2, off : off + 512],
                        pr[:],
                        xbt[:, 0:2, hw0 : hw0 + 512],
                        op=AL.add,
                    )
                    nc.scalar.copy(stage[:, 2:4, off : off + 512], pe[:])
                # one descriptor per block via a [128, 4, win] dram view
                # (dim0 = partition, dim1 = oc chunk); last block DMA'd
                # per tile to shorten the pipeline-drain tail.
                outv = out_d.rearrange("(o p) w -> p o w", o=CK)
                if g == NBLK - 1:
                    hw1 = g * BLK + off
                    nc.gpsimd.dma_start(
                        out=outv[:, :, hw1 : hw1 + 512],
                        in_=stage[:, :, off : off + 512],
                    )
                elif ti == 2:
                    nc.gpsimd.dma_start(
                        out=outv[:, :, g * BLK : (g + 1) * BLK],
                        in_=stage[:],
                    )


def _build(use_bias):
    nc = bacc.Bacc("TRN2", target_bir_lowering=False, debug=False, num_devices=N_CORES)
    aps = {
        "xq": nc.dram_tensor("xq", [C, HW], FP8, kind="ExternalInput").ap(),
        "xb": nc.dram_tensor("xb", [C, HW], BF16, kind="ExternalInput").ap(),
        "w12q": nc.dram_tensor("w12q", [C, KM], FP8, kind="ExternalInput").ap(),
        "w3t": nc.dram_tensor("w3t", [C4, C4], BF16, kind="ExternalInput").ap(),
        "w4t": nc.dram_tensor("w4t", [C2, C2], BF16, kind="ExternalInput").ap(),
        "w5t": nc.dram_tensor("w5t", [C2, C], BF16, kind="ExternalInput").ap(),
        "identb": nc.dram_tensor(
            "identb", [128, 128], BF16, kind="ExternalInput"
        ).ap(),
        "ident8": nc.dram_tensor(
            "ident8", [128, 128], FP8, kind="ExternalInput"
        ).ap(),
        "out": nc.dram_tensor("out", [C, HW], BF16, kind="ExternalOutput").ap(),
    }
    if use_bias:
        aps["b12row"] = nc.dram_tensor(
            "b12row", [1, KM], F32, kind="ExternalInput"
        ).ap()
        aps["b3c"] = nc.dram_tensor("b3c", [C4, 1], F32, kind="ExternalInput").ap()
        aps["b4c"] = nc.dram_tensor("b4c", [C2, 1], F32, kind="ExternalInput").ap()
        aps["b5c"] = nc.dram_tensor("b5c", [C, 1], F32, kind="ExternalInput").ap()

    from contextlib import ExitStack

    with tile.TileContext(nc) as tc:
        with ExitStack() as ctx:
            _emit(ctx, tc, aps, use_bias)
    nc.compile()
    return nc


_CACHE = {}


def _run(inputs, trace=False, **run_kwargs):
    x = np.ascontiguousarray(np.asarray(inputs["x"], dtype=np.float32))
    assert x.shape == (N_CORES, C, H, W_IMG), x.shape
    w1 = np.asarray(inputs["w1"], dtype=np.float32)
    w2 = np.asarray(inputs["w2"], dtype=np.float32)
    w3 = np.asarray(inputs["w3"], dtype=np.float32)
    w4 = np.asarray(inputs["w4"], dtype=np.float32)
    w5 = np.asarray(inputs["w5"], dtype=np.float32)
    b1 = np.asarray(inputs["b1"], dtype=np.float32)
    b2 = np.asarray(inputs["b2"], dtype=np.float32)
    b3 = np.asarray(inputs["b3"], dtype=np.float32)
    b4 = np.asarray(inputs["b4"], dtype=np.float32)
    b5 = np.asarray(inputs["b5"], dtype=np.float32)
    use_bias = bool(
        np.any(b1) or np.any(b2) or np.any(b3) or np.any(b4) or np.any(b5)
    )

    if use_bias not in _CACHE:
        _CACHE[use_bias] = _build(use_bias)
    nc = _CACHE[use_bias]

    BF = ml_dtypes.bfloat16
    E4 = ml_dtypes.float8_e4m3
    w12t = np.concatenate([w1.T, w2.T], axis=1)  # [512, 384]
    shared = {
        "w12q": np.ascontiguousarray((w12t * WSCALE).astype(E4)),
        "w3t": np.ascontiguousarray(w3.T.astype(BF)),
        "w4t": np.ascontiguousarray(w4.T.astype(BF)),
        "w5t": np.ascontiguousarray(w5.T.astype(BF)),
        "identb": np.eye(128, dtype=BF),
        "ident8": np.eye(128, dtype=E4),
    }
    if use_bias:
        shared["b12row"] = np.ascontiguousarray(
            (np.concatenate([b1, b2]) * WSCALE)[None, :], dtype=np.float32
        )
        shared["b3c"] = np.ascontiguousarray(b3[:, None])
        shared["b4c"] = np.ascontiguousarray(b4[:, None])
        shared["b5c"] = np.ascontiguousarray(b5[:, None])

    xr = x.reshape(N_CORES, C, HW)
    in_maps = [
        {
            "xq": np.ascontiguousarray(xr[b].astype(E4)),
            "xb": np.ascontiguousarray(xr[b].astype(BF)),
            **shared,
        }
        for b in range(N_CORES)
    ]
    res = run_bass_kernel_spmd(
        nc, in_maps, core_ids=list(range(N_CORES)), trace=trace, **run_kwargs
    )
    out = np.stack(
        [
            res.results[b]["out"].astype(np.float32).reshape(C, H, W_IMG)
            for b in range(N_CORES)
        ]
    )
    return out, res


def kernel(**inputs):
    out, _ = _run(inputs, trace=False)
    return out
